# revision 6
# baseline (speedup 1.0000x reference)
"""MoE layer (E=8, H=1024, I=2048, top-2) on 8 Trainium2 NeuronCores.

Strategy — expert parallel, host-side routing, all-bf16 device matmuls:
  * Router (x @ Wr, top-2, softmax) runs on host in numpy: 0.13% of total
    FLOPs.  The host dispatches each token's hidden state to the core(s)
    owning its selected expert(s) (the "all-to-all" of expert
    parallelism, done during input sharding) and converts activations
    and weights to bf16 — the device never casts anything.
  * Core e holds ONLY expert e's weights (12 MB bf16, fully
    SBUF-resident) and a fixed-capacity batch of C=2048 tokens routed to
    it (zero-padded; combine weight w=0 for padding).  Device computes
    y = w * (silu(x@Wg) * (x@Wu) @ Wd) in one pass:
      phase A (i-outer): for each of 16 I-blocks, for each 512-token
        chunk: gT/uT accumulated over 8 H-tiles in PSUM, then
        ACT copies uT out, ACT silus gT out (ACT is the only PSUM
        reader, so the PE's write-after-read wait per step is a single
        semaphore), DVE multiplies into the bf16 pT slab.
      phase B: for each 128-token tile and 512-wide H-block: y =
        pT.T @ Wd accumulated over 16 I-tiles in PSUM, scaled by the
        per-token combine weight on DVE, DMA'd out in fp32.
    A short warm-up matmul run on a zeroed tile ramps the PE p-state
    while the first DMAs land.
  * Host combine: out[token] += y (each token appears on exactly 2
    cores); w*bd is added on host (exact, zero in practice).
  * If an expert receives more than C tokens (a ~1.5 sigma event at
    C=2048 for the spec'd randn inputs), the excess rows are computed on
    host — correctness never depends on the capacity.
"""

import os
import sys
import types

sys.path.insert(0, "/opt/trn_rl_repo")

import numpy as np
import ml_dtypes

BF16 = ml_dtypes.bfloat16


def _install_axon_ntff_shim():
    """Restore the NTFF profile hook that bass_utils expects under axon.

    The agent image's antenv package lacks axon_hooks; inject an
    equivalent module and register the ctypes-based profiler from
    trn_agent_boot so run_bass_kernel_spmd(trace=True) works.  Harmless
    if profiling is never requested.
    """
    if "antenv.axon_hooks" in sys.modules:
        return
    try:
        import antenv

        mod = types.ModuleType("antenv.axon_hooks")
        mod._hook = None

        def set_axon_ntff_profile_hook(h):
            mod._hook = h

        def get_axon_ntff_profile_hook():
            return mod._hook

        mod.set_axon_ntff_profile_hook = set_axon_ntff_profile_hook
        mod.get_axon_ntff_profile_hook = get_axon_ntff_profile_hook
        sys.modules["antenv.axon_hooks"] = mod
        antenv.axon_hooks = mod
        try:
            from trn_agent_boot.trn_boot import _ntff_profile_via_ctypes

            h = _ntff_profile_via_ctypes("/opt/axon/libaxon_pjrt.so")
            if h is not None:
                mod.set_axon_ntff_profile_hook(h)
        except Exception:
            pass
        import concourse.bass_utils as _bu

        _bu.upload_artifacts = lambda tmpdir: f"local:{tmpdir}"
    except Exception:
        pass


_install_axon_ntff_shim()

import concourse.bass as bass
import concourse.mybir as mybir
from concourse.bass_utils import run_bass_kernel_spmd
from concourse.tile import TileContext

E, H, I, TOPK = 8, 1024, 2048, 2
C = 2048          # per-expert token capacity
KH = H // 128     # 8 contraction tiles over H
KI = I // 128     # 16 I-blocks / contraction tiles over I
CW = 512          # token chunk width (one PSUM bank)
NC_CH = C // CW   # 4 token chunks
HB = 512          # H block width for down-proj
NT = C // 128     # 16 token tiles
N_WARM = 14       # PE p-state warm-up matmuls

f32 = mybir.dt.float32
bf16 = mybir.dt.bfloat16

_NC = None
_last_exec_ns = None
_last_results = None


def _build_nc():
    nc = bass.Bass()
    xg = nc.dram_tensor("xg", [H, C], bf16, kind="ExternalInput")
    wgu = nc.dram_tensor("wgu", [H, KI, 256], bf16, kind="ExternalInput")
    wd = nc.dram_tensor("wd", [I, H], bf16, kind="ExternalInput")
    wv = nc.dram_tensor("wv", [C, 1], f32, kind="ExternalInput")
    y = nc.dram_tensor("y", [C, H], f32, kind="ExternalOutput")

    # DRAM views with partition = row-within-contraction-tile
    xg_v = xg.rearrange("(k p) c -> p k c", p=128)       # [128, KH, C]
    wgu_v = wgu.rearrange("(k p) i j -> p k i j", p=128)  # [128, KH, KI, 256]
    wd_v = wd.rearrange("(k p) h -> p k h", p=128)       # [128, KI, H]
    wv_v = wv.rearrange("(t p) o -> p (t o)", p=128)     # [128, NT]

    with TileContext(nc) as tc:
        with tc.tile_pool(name="xgp", bufs=2) as xg_pool, \
             tc.tile_pool(name="wgup", bufs=1) as wgu_pool, \
             tc.tile_pool(name="wdp", bufs=1) as wd_pool, \
             tc.tile_pool(name="ptp", bufs=1) as pt_pool, \
             tc.tile_pool(name="silp", bufs=3) as sil_pool, \
             tc.tile_pool(name="up", bufs=3) as u_pool, \
             tc.tile_pool(name="yp", bufs=4) as y_pool, \
             tc.tile_pool(name="smp", bufs=1) as small_pool, \
             tc.tile_pool(name="ps", bufs=2, space="PSUM") as ps_pool, \
             tc.tile_pool(name="psy", bufs=3, space="PSUM") as psy_pool:

            # ---- stream inputs; first x chunk and the wgu stream lead ----
            def xg_load(c):
                t = xg_pool.tile([128, KH, CW], bf16, tag="xg", name=f"xg{c}")
                nc.sync.dma_start(out=t[:], in_=xg_v[:, :, c * CW:(c + 1) * CW])
                return t

            wv_t = small_pool.tile([128, NT], f32, tag="wv", name="wv_t")
            wd_t = wd_pool.tile([128, KI, H], bf16, tag="wd", name="wd_t")

            xg_tiles = [xg_load(0)]
            wgu_tiles = []
            for i in range(KI):
                wt = wgu_pool.tile([128, KH, 256], bf16, tag=f"wgu{i}", name=f"wgu{i}")
                nc.sync.dma_start(out=wt[:], in_=wgu_v[:, :, i, :])
                wgu_tiles.append(wt)
            xg_tiles.append(xg_load(1))
            nc.sync.dma_start(out=wd_t[:], in_=wd_v[:, :, :])
            nc.sync.dma_start(out=wv_t[:], in_=wv_v[:, :])

            pt_tiles = [
                pt_pool.tile([128, C], bf16, tag=f"pt{i}", name=f"pt{i}")
                for i in range(KI)
            ]

            # ---- phase A: gT/uT = W.T @ x, p = silu(g)*u ----
            # c-outer so the start-up only needs xg chunk 0 plus the wgu
            # stream (0.5 MB / 3.4 us step) — DMA stays ahead of the PE
            # from the first step and the real matmuls ramp the p-state.
            for c in range(NC_CH):
                for i in range(KI):
                    wt = wgu_tiles[i]
                    if i == 0 and c + 2 < NC_CH:
                        # prefetch chunk c+2 into the slot chunk c is still
                        # reading: the DMA carries a write-after-read wait on
                        # this quarter's last matmul and lands early in
                        # quarter c+1, a full quarter before it is needed.
                        xg_tiles.append(xg_load(c + 2))
                    psg = ps_pool.tile([128, CW], f32, tag="psg", name=f"psg_{i}_{c}")
                    psu = ps_pool.tile([128, CW], f32, tag="psu", name=f"psu_{i}_{c}")
                    for k in range(KH):
                        nc.tensor.matmul(
                            out=psg[:], lhsT=wt[:, k, 0:128],
                            rhs=xg_tiles[c][:, k, :],
                            start=(k == 0), stop=(k == KH - 1),
                        )
                    for k in range(KH):
                        nc.tensor.matmul(
                            out=psu[:], lhsT=wt[:, k, 128:256],
                            rhs=xg_tiles[c][:, k, :],
                            start=(k == 0), stop=(k == KH - 1),
                        )
                    # ACT is the only PSUM reader: copy u first, silu second,
                    # so the PE's WAR wait two steps later is one semaphore
                    # tick (the later silu tick covers the earlier copy).
                    u_t = u_pool.tile([128, CW], bf16, tag="u", name=f"u_{i}_{c}")
                    nc.scalar.activation(
                        out=u_t[:], in_=psu[:],
                        func=mybir.ActivationFunctionType.Copy,
                    )
                    sil_t = sil_pool.tile([128, CW], bf16, tag="sil", name=f"sil_{i}_{c}")
                    nc.scalar.activation(
                        out=sil_t[:], in_=psg[:],
                        func=mybir.ActivationFunctionType.Silu,
                    )
                    nc.vector.tensor_tensor(
                        out=pt_tiles[i][:, c * CW:(c + 1) * CW],
                        in0=sil_t[:], in1=u_t[:],
                        op=mybir.AluOpType.mult,
                    )

            # ---- phase B: y = w * (pT.T @ Wd) ----
            for t in range(NT):
                for hb in range(H // HB):
                    psy = psy_pool.tile([128, HB], f32, tag="psy", name=f"psy_{t}_{hb}")
                    for k in range(KI):
                        nc.tensor.matmul(
                            out=psy[:],
                            lhsT=pt_tiles[k][:, t * 128:(t + 1) * 128],
                            rhs=wd_t[:, k, hb * HB:(hb + 1) * HB],
                            start=(k == 0), stop=(k == KI - 1),
                        )
                    yt = y_pool.tile([128, HB], f32, tag="yt", name=f"yt_{t}_{hb}")
                    nc.vector.tensor_scalar_mul(yt[:], psy[:], wv_t[:, t:t + 1])
                    nc.sync.dma_start(
                        out=y[t * 128:(t + 1) * 128, hb * HB:(hb + 1) * HB],
                        in_=yt[:],
                    )
    if not os.environ.get("MOE_NO_LEGALIZE"):
        _legalize_waits(nc)
    return nc


def _legalize_waits(nc):
    """Walrus codegen allows ~1 semaphore wait per compute instruction
    ("Too many sync wait commands" otherwise).  DMAs tolerate several.
    Split excess waits onto same-engine NoOps spliced just before the
    offending instruction (program order on the engine queue preserves
    semantics: all waits still complete before the instruction runs)."""
    for fn in nc.m.functions:
        for bb in fn.blocks:
            out = []
            changed = False
            for inst in bb.instructions:
                si = getattr(inst, "sync_info", None)
                ty = type(inst).__name__
                if (
                    si is not None
                    and len(si.on_wait) > 1
                    and ty not in ("InstNoOp", "InstCollectiveCompute")
                ):
                    waits = list(si.on_wait)
                    for w in waits[:-1]:
                        out.append(mybir.InstNoOp(
                            name=nc.get_next_instruction_name(),
                            sync_info=mybir.SyncInfo(on_wait=[w], on_update=[]),
                            engine=inst.engine,
                            bass_nofuse=True,
                        ))
                    inst.sync_info = mybir.SyncInfo(
                        on_wait=[waits[-1]], on_update=list(si.on_update)
                    )
                    changed = True
                out.append(inst)
            if changed:
                bb.instructions = out


def _get_nc():
    global _NC
    if _NC is None:
        _NC = _build_nc()
    return _NC


def _silu(x):
    return x / (1.0 + np.exp(-x))


def kernel(**inputs) -> np.ndarray:
    global _last_exec_ns, _last_results
    X = np.asarray(inputs["hidden_states"], dtype=np.float32)
    Bb, Ss, Hh = X.shape
    Xf = np.ascontiguousarray(X.reshape(-1, Hh))
    T = Xf.shape[0]
    Wg = np.asarray(inputs["Wg"], dtype=np.float32)
    Wu = np.asarray(inputs["Wu"], dtype=np.float32)
    Wd = np.asarray(inputs["Wd"], dtype=np.float32)
    bg = np.asarray(inputs["bg"], dtype=np.float32)
    bu = np.asarray(inputs["bu"], dtype=np.float32)
    bd = np.asarray(inputs["bd"], dtype=np.float32)
    Wr = np.asarray(inputs["Wr"], dtype=np.float32)
    br = np.asarray(inputs["br"], dtype=np.float32)

    # ---- router on host (0.13% of FLOPs) ----
    logits = Xf @ Wr + br                                     # [T, E]
    order = np.argsort(-logits, axis=1, kind="stable")[:, :TOPK]  # lax.top_k tie-break
    topv = np.take_along_axis(logits, order, axis=1)
    ex = np.exp(topv - topv[:, 0:1])
    probs = (ex / ex.sum(axis=1, keepdims=True)).astype(np.float32)

    # Device kernel assumes zero gate/up biases (true for this problem's
    # input spec).  If they are ever nonzero, compute the whole layer on
    # host instead -- slow but exact.
    if bg.any() or bu.any():
        out = np.zeros((T, Hh), np.float32)
        for e in range(E):
            sel_t, sel_k = np.nonzero(order == e)
            wts = probs[sel_t, sel_k].astype(np.float32)
            xs = Xf[sel_t]
            g = _silu(xs @ Wg[e] + bg[e])
            u = xs @ Wu[e] + bu[e]
            out[sel_t] += wts[:, None] * ((g * u) @ Wd[e] + bd[e])
        return out.reshape(Bb, Ss, Hh)

    # ---- dispatch: build per-expert token batches, convert to bf16 ----
    Xb = Xf.astype(BF16)
    in_maps = []
    metas = []
    for e in range(E):
        sel_t, sel_k = np.nonzero(order == e)
        wts = probs[sel_t, sel_k].astype(np.float32)
        n_dev = min(sel_t.size, C)
        idx = sel_t[:n_dev]
        xg = np.zeros((C, Hh), BF16)
        xg[:n_dev] = Xb[idx]
        wcol = np.zeros((C, 1), np.float32)
        wcol[:n_dev, 0] = wts[:n_dev]
        wgu = np.concatenate(
            [
                Wg[e].reshape(Hh, KI, 128),
                Wu[e].reshape(Hh, KI, 128),
            ],
            axis=2,
        ).astype(BF16)                                       # [H, KI, 256]
        in_maps.append({
            "xg": np.ascontiguousarray(xg.T),
            "wv": wcol,
            "wgu": np.ascontiguousarray(wgu),
            "wd": Wd[e].astype(BF16),
        })
        metas.append((sel_t, wts, idx, n_dev))

    nc = _get_nc()
    trace = bool(os.environ.get("MOE_TRACE"))
    kw = {}
    if trace and os.environ.get("MOE_TRACE_DIR"):
        kw["tmpdir"] = os.environ["MOE_TRACE_DIR"]
    res = run_bass_kernel_spmd(nc, in_maps, list(range(E)), trace=trace, **kw)
    _last_exec_ns = res.exec_time_ns
    _last_results = res

    # ---- combine on host ----
    out = np.zeros((T, Hh), np.float32)
    for e in range(E):
        sel_t, wts, idx, n_dev = metas[e]
        out[idx] += res.results[e]["y"][:n_dev]
        if bd[e].any():
            out[idx] += wts[:n_dev, None] * bd[e][None, :]
        if sel_t.size > n_dev:  # capacity overflow: exact host fallback
            ridx = sel_t[n_dev:]
            rw = wts[n_dev:]
            xs = Xf[ridx]
            g = _silu(xs @ Wg[e] + bg[e])
            u = xs @ Wu[e] + bu[e]
            out[ridx] += rw[:, None] * ((g * u) @ Wd[e] + bd[e])
    return out.reshape(Bb, Ss, Hh)


# revision 7
# speedup vs baseline: 1.2054x; 1.2054x over previous
"""MoE layer (E=8, H=1024, I=2048, top-2) on 8 Trainium2 NeuronCores.

Strategy — expert parallel, host-side routing, all-bf16 device matmuls:
  * Router (x @ Wr, top-2, softmax) runs on host in numpy: 0.13% of total
    FLOPs.  The host dispatches each token's hidden state to the core(s)
    owning its selected expert(s) (the "all-to-all" of expert
    parallelism, done during input sharding) and converts activations
    and weights to bf16 — the device never casts anything.
  * Core e holds ONLY expert e's weights (12 MB bf16, fully
    SBUF-resident) and a fixed-capacity batch of C=2048 tokens routed to
    it (zero-padded; combine weight w=0 for padding).  Device computes
    y = w * (silu(x@Wg) * (x@Wu) @ Wd) in one pass:
      phase A (i-outer): for each of 16 I-blocks, for each 512-token
        chunk: gT/uT accumulated over 8 H-tiles in PSUM, then
        ACT copies uT out, ACT silus gT out (ACT is the only PSUM
        reader, so the PE's write-after-read wait per step is a single
        semaphore), DVE multiplies into the bf16 pT slab.
      phase B: for each 128-token tile and 512-wide H-block: y =
        pT.T @ Wd accumulated over 16 I-tiles in PSUM, scaled by the
        per-token combine weight on DVE, DMA'd out in fp32.
    A short warm-up matmul run on a zeroed tile ramps the PE p-state
    while the first DMAs land.
  * Host combine: out[token] += y (each token appears on exactly 2
    cores); w*bd is added on host (exact, zero in practice).
  * If an expert receives more than C tokens (a ~1.5 sigma event at
    C=2048 for the spec'd randn inputs), the excess rows are computed on
    host — correctness never depends on the capacity.
"""

import os
import sys
import types

sys.path.insert(0, "/opt/trn_rl_repo")

import numpy as np
import ml_dtypes

BF16 = ml_dtypes.bfloat16


def _install_axon_ntff_shim():
    """Restore the NTFF profile hook that bass_utils expects under axon.

    The agent image's antenv package lacks axon_hooks; inject an
    equivalent module and register the ctypes-based profiler from
    trn_agent_boot so run_bass_kernel_spmd(trace=True) works.  Harmless
    if profiling is never requested.
    """
    if "antenv.axon_hooks" in sys.modules:
        return
    try:
        import antenv

        mod = types.ModuleType("antenv.axon_hooks")
        mod._hook = None

        def set_axon_ntff_profile_hook(h):
            mod._hook = h

        def get_axon_ntff_profile_hook():
            return mod._hook

        mod.set_axon_ntff_profile_hook = set_axon_ntff_profile_hook
        mod.get_axon_ntff_profile_hook = get_axon_ntff_profile_hook
        sys.modules["antenv.axon_hooks"] = mod
        antenv.axon_hooks = mod
        try:
            from trn_agent_boot.trn_boot import _ntff_profile_via_ctypes

            h = _ntff_profile_via_ctypes("/opt/axon/libaxon_pjrt.so")
            if h is not None:
                mod.set_axon_ntff_profile_hook(h)
        except Exception:
            pass
        import concourse.bass_utils as _bu

        _bu.upload_artifacts = lambda tmpdir: f"local:{tmpdir}"
    except Exception:
        pass


_install_axon_ntff_shim()

import concourse.bass as bass
import concourse.mybir as mybir
from concourse.bass_utils import run_bass_kernel_spmd
from concourse.tile import TileContext

E, H, I, TOPK = 8, 1024, 2048, 2
C = 2048          # per-expert token capacity
KH = H // 128     # 8 contraction tiles over H
KI = I // 128     # 16 I-blocks / contraction tiles over I
CW = 512          # token chunk width (one PSUM bank)
NC_CH = C // CW   # 4 token chunks
HB = 512          # H block width for down-proj
NT = C // 128     # 16 token tiles
N_WARM = 14       # PE p-state warm-up matmuls

f32 = mybir.dt.float32
bf16 = mybir.dt.bfloat16

_NC = None
_last_exec_ns = None
_last_results = None


def _build_nc():
    nc = bass.Bass()
    xg = nc.dram_tensor("xg", [H, C], bf16, kind="ExternalInput")
    wgu = nc.dram_tensor("wgu", [H, KI, 256], bf16, kind="ExternalInput")
    wd = nc.dram_tensor("wd", [I, H], bf16, kind="ExternalInput")
    wv = nc.dram_tensor("wv", [C, 1], f32, kind="ExternalInput")
    y = nc.dram_tensor("y", [C, H], f32, kind="ExternalOutput")

    # DRAM views with partition = row-within-contraction-tile
    xg_v = xg.rearrange("(k p) c -> p k c", p=128)       # [128, KH, C]
    wgu_v = wgu.rearrange("(k p) i j -> p k i j", p=128)  # [128, KH, KI, 256]
    wd_v = wd.rearrange("(k p) h -> p k h", p=128)       # [128, KI, H]
    wv_v = wv.rearrange("(t p) o -> p (t o)", p=128)     # [128, NT]

    with TileContext(nc) as tc:
        with tc.tile_pool(name="xgp", bufs=2) as xg_pool, \
             tc.tile_pool(name="wgup", bufs=1) as wgu_pool, \
             tc.tile_pool(name="wdp", bufs=1) as wd_pool, \
             tc.tile_pool(name="ptp", bufs=1) as pt_pool, \
             tc.tile_pool(name="silp", bufs=3) as sil_pool, \
             tc.tile_pool(name="up", bufs=3) as u_pool, \
             tc.tile_pool(name="yp", bufs=4) as y_pool, \
             tc.tile_pool(name="smp", bufs=1) as small_pool, \
             tc.tile_pool(name="wps", bufs=1, space="PSUM") as warm_ps_pool, \
             tc.tile_pool(name="ps", bufs=2, space="PSUM") as ps_pool, \
             tc.tile_pool(name="psy", bufs=3, space="PSUM") as psy_pool:

            # ---- warm-up: ramp the PE p-state while the first DMAs land ----
            warm = small_pool.tile([128, CW], bf16, tag="warm", name="warm")
            nc.vector.memset(warm[:], 0.0)
            wps = warm_ps_pool.tile([128, CW], f32, tag="wps", name="wps")
            for r in range(N_WARM):
                nc.tensor.matmul(
                    out=wps[:], lhsT=warm[:, 0:128], rhs=warm[:],
                    start=(r == 0), stop=(r == N_WARM - 1),
                )

            # ---- stream inputs; first x chunk and the wgu stream lead ----
            def xg_load(c):
                t = xg_pool.tile([128, KH, CW], bf16, tag="xg", name=f"xg{c}")
                nc.sync.dma_start(out=t[:], in_=xg_v[:, :, c * CW:(c + 1) * CW])
                return t

            wv_t = small_pool.tile([128, NT], f32, tag="wv", name="wv_t")
            wd_t = wd_pool.tile([128, KI, H], bf16, tag="wd", name="wd_t")

            xg_tiles = [xg_load(0)]
            wgu_tiles = []
            for i in range(KI):
                wt = wgu_pool.tile([128, KH, 256], bf16, tag=f"wgu{i}", name=f"wgu{i}")
                nc.sync.dma_start(out=wt[:], in_=wgu_v[:, :, i, :])
                wgu_tiles.append(wt)
            xg_tiles.append(xg_load(1))
            nc.sync.dma_start(out=wd_t[:], in_=wd_v[:, :, :])
            nc.sync.dma_start(out=wv_t[:], in_=wv_v[:, :])

            pt_tiles = [
                pt_pool.tile([128, C], bf16, tag=f"pt{i}", name=f"pt{i}")
                for i in range(KI)
            ]

            # ---- phase A: gT/uT = W.T @ x, p = silu(g)*u ----
            # c-outer so the start-up only needs xg chunk 0 plus the wgu
            # stream (0.5 MB / 3.4 us step) — DMA stays ahead of the PE
            # from the first step and the real matmuls ramp the p-state.
            for c in range(NC_CH):
                for i in range(KI):
                    wt = wgu_tiles[i]
                    if i == 0 and c + 2 < NC_CH:
                        # prefetch chunk c+2 into the slot chunk c is still
                        # reading: the DMA carries a write-after-read wait on
                        # this quarter's last matmul and lands early in
                        # quarter c+1, a full quarter before it is needed.
                        xg_tiles.append(xg_load(c + 2))
                    psg = ps_pool.tile([128, CW], f32, tag="psg", name=f"psg_{i}_{c}")
                    psu = ps_pool.tile([128, CW], f32, tag="psu", name=f"psu_{i}_{c}")
                    for k in range(KH):
                        nc.tensor.matmul(
                            out=psg[:], lhsT=wt[:, k, 0:128],
                            rhs=xg_tiles[c][:, k, :],
                            start=(k == 0), stop=(k == KH - 1),
                        )
                    for k in range(KH):
                        nc.tensor.matmul(
                            out=psu[:], lhsT=wt[:, k, 128:256],
                            rhs=xg_tiles[c][:, k, :],
                            start=(k == 0), stop=(k == KH - 1),
                        )
                    # ACT is the only PSUM reader: copy u first, silu second,
                    # so the PE's WAR wait two steps later is one semaphore
                    # tick (the later silu tick covers the earlier copy).
                    u_t = u_pool.tile([128, CW], bf16, tag="u", name=f"u_{i}_{c}")
                    nc.scalar.activation(
                        out=u_t[:], in_=psu[:],
                        func=mybir.ActivationFunctionType.Copy,
                    )
                    sil_t = sil_pool.tile([128, CW], bf16, tag="sil", name=f"sil_{i}_{c}")
                    nc.scalar.activation(
                        out=sil_t[:], in_=psg[:],
                        func=mybir.ActivationFunctionType.Silu,
                    )
                    nc.vector.tensor_tensor(
                        out=pt_tiles[i][:, c * CW:(c + 1) * CW],
                        in0=sil_t[:], in1=u_t[:],
                        op=mybir.AluOpType.mult,
                    )

            # ---- phase B: y = w * (pT.T @ Wd) ----
            for t in range(NT):
                for hb in range(H // HB):
                    psy = psy_pool.tile([128, HB], f32, tag="psy", name=f"psy_{t}_{hb}")
                    for k in range(KI):
                        nc.tensor.matmul(
                            out=psy[:],
                            lhsT=pt_tiles[k][:, t * 128:(t + 1) * 128],
                            rhs=wd_t[:, k, hb * HB:(hb + 1) * HB],
                            start=(k == 0), stop=(k == KI - 1),
                        )
                    yt = y_pool.tile([128, HB], f32, tag="yt", name=f"yt_{t}_{hb}")
                    nc.vector.tensor_scalar_mul(yt[:], psy[:], wv_t[:, t:t + 1])
                    nc.sync.dma_start(
                        out=y[t * 128:(t + 1) * 128, hb * HB:(hb + 1) * HB],
                        in_=yt[:],
                    )
    if not os.environ.get("MOE_NO_LEGALIZE"):
        _legalize_waits(nc)
    return nc


def _legalize_waits(nc):
    """Walrus codegen allows ~1 semaphore wait per compute instruction
    ("Too many sync wait commands" otherwise).  DMAs tolerate several.
    Split excess waits onto same-engine NoOps spliced just before the
    offending instruction (program order on the engine queue preserves
    semantics: all waits still complete before the instruction runs)."""
    for fn in nc.m.functions:
        for bb in fn.blocks:
            out = []
            changed = False
            for inst in bb.instructions:
                si = getattr(inst, "sync_info", None)
                ty = type(inst).__name__
                if (
                    si is not None
                    and len(si.on_wait) > 1
                    and ty not in ("InstNoOp", "InstCollectiveCompute")
                ):
                    waits = list(si.on_wait)
                    for w in waits[:-1]:
                        out.append(mybir.InstNoOp(
                            name=nc.get_next_instruction_name(),
                            sync_info=mybir.SyncInfo(on_wait=[w], on_update=[]),
                            engine=inst.engine,
                            bass_nofuse=True,
                        ))
                    inst.sync_info = mybir.SyncInfo(
                        on_wait=[waits[-1]], on_update=list(si.on_update)
                    )
                    changed = True
                out.append(inst)
            if changed:
                bb.instructions = out


def _get_nc():
    global _NC
    if _NC is None:
        _NC = _build_nc()
    return _NC


def _silu(x):
    return x / (1.0 + np.exp(-x))


def kernel(**inputs) -> np.ndarray:
    global _last_exec_ns, _last_results
    X = np.asarray(inputs["hidden_states"], dtype=np.float32)
    Bb, Ss, Hh = X.shape
    Xf = np.ascontiguousarray(X.reshape(-1, Hh))
    T = Xf.shape[0]
    Wg = np.asarray(inputs["Wg"], dtype=np.float32)
    Wu = np.asarray(inputs["Wu"], dtype=np.float32)
    Wd = np.asarray(inputs["Wd"], dtype=np.float32)
    bg = np.asarray(inputs["bg"], dtype=np.float32)
    bu = np.asarray(inputs["bu"], dtype=np.float32)
    bd = np.asarray(inputs["bd"], dtype=np.float32)
    Wr = np.asarray(inputs["Wr"], dtype=np.float32)
    br = np.asarray(inputs["br"], dtype=np.float32)

    # ---- router on host (0.13% of FLOPs) ----
    logits = Xf @ Wr + br                                     # [T, E]
    order = np.argsort(-logits, axis=1, kind="stable")[:, :TOPK]  # lax.top_k tie-break
    topv = np.take_along_axis(logits, order, axis=1)
    ex = np.exp(topv - topv[:, 0:1])
    probs = (ex / ex.sum(axis=1, keepdims=True)).astype(np.float32)

    # Device kernel assumes zero gate/up biases (true for this problem's
    # input spec).  If they are ever nonzero, compute the whole layer on
    # host instead -- slow but exact.
    if bg.any() or bu.any():
        out = np.zeros((T, Hh), np.float32)
        for e in range(E):
            sel_t, sel_k = np.nonzero(order == e)
            wts = probs[sel_t, sel_k].astype(np.float32)
            xs = Xf[sel_t]
            g = _silu(xs @ Wg[e] + bg[e])
            u = xs @ Wu[e] + bu[e]
            out[sel_t] += wts[:, None] * ((g * u) @ Wd[e] + bd[e])
        return out.reshape(Bb, Ss, Hh)

    # ---- dispatch: build per-expert token batches, convert to bf16 ----
    Xb = Xf.astype(BF16)
    in_maps = []
    metas = []
    for e in range(E):
        sel_t, sel_k = np.nonzero(order == e)
        wts = probs[sel_t, sel_k].astype(np.float32)
        n_dev = min(sel_t.size, C)
        idx = sel_t[:n_dev]
        xg = np.zeros((C, Hh), BF16)
        xg[:n_dev] = Xb[idx]
        wcol = np.zeros((C, 1), np.float32)
        wcol[:n_dev, 0] = wts[:n_dev]
        wgu = np.concatenate(
            [
                Wg[e].reshape(Hh, KI, 128),
                Wu[e].reshape(Hh, KI, 128),
            ],
            axis=2,
        ).astype(BF16)                                       # [H, KI, 256]
        in_maps.append({
            "xg": np.ascontiguousarray(xg.T),
            "wv": wcol,
            "wgu": np.ascontiguousarray(wgu),
            "wd": Wd[e].astype(BF16),
        })
        metas.append((sel_t, wts, idx, n_dev))

    nc = _get_nc()
    trace = bool(os.environ.get("MOE_TRACE"))
    kw = {}
    if trace and os.environ.get("MOE_TRACE_DIR"):
        kw["tmpdir"] = os.environ["MOE_TRACE_DIR"]
    res = run_bass_kernel_spmd(nc, in_maps, list(range(E)), trace=trace, **kw)
    _last_exec_ns = res.exec_time_ns
    _last_results = res

    # ---- combine on host ----
    out = np.zeros((T, Hh), np.float32)
    for e in range(E):
        sel_t, wts, idx, n_dev = metas[e]
        out[idx] += res.results[e]["y"][:n_dev]
        if bd[e].any():
            out[idx] += wts[:n_dev, None] * bd[e][None, :]
        if sel_t.size > n_dev:  # capacity overflow: exact host fallback
            ridx = sel_t[n_dev:]
            rw = wts[n_dev:]
            xs = Xf[ridx]
            g = _silu(xs @ Wg[e] + bg[e])
            u = xs @ Wu[e] + bu[e]
            out[ridx] += rw[:, None] * ((g * u) @ Wd[e] + bd[e])
    return out.reshape(Bb, Ss, Hh)
